# revision 9
# baseline (speedup 1.0000x reference)
"""Trainium2 Bass kernel for the MinGRU (full-GRU) problem.

Shapes (hardcoded): x [256, 512, 256], W*_w [768, 512], W*_b [512],
fc_w [512, 10], fc_b [10, 1].  Output [256, 10] fp32.

Strategy: data-parallel over batch across 8 cores (B_local = 32).
Per core, everything lives in a transposed tile layout
(partition = feature-within-128-chunk, column = 32*chunk + batch):

  Phase 1 (interleaved): U_g = x @ W_g[H:] + b_g for all t, computed
      chunk-by-chunk (16 timesteps each) directly into an SBUF ring —
      no DRAM roundtrip.  Its matmuls/activations are emitted into the
      tail of each recurrence step so they fill engine idle slots.

  Recurrence (per step), scheduled to minimize the serial chain
      sigma(r) -> r*h -> h_tilde matmul -> tanh -> blend:
      - r-gate matmuls first, split across two PSUM banks so the first
        sigmoid (chunks 0,1) overlaps the second half's matmuls;
      - r*h computed per half, h_tilde matmuls start as soon as the
        k=0,1 chunks of r*h exist;
      - z-gate matmuls (single bank, single sigmoid) run in the shadow
        of the sigma(r)/r*h chain; (1-z)*h is precomputed off-path;
      - single tanh over the whole h_tilde bank; the blend
        h = (h - z*h) + z*h_tilde is split in column halves so the next
        step's k=0,1 matmuls can start before the second half lands.
"""

import os
import sys
import threading

import numpy as np

sys.path.insert(0, "/opt/trn_rl_repo")

import ml_dtypes

BF16 = ml_dtypes.bfloat16

B, T, F, H, C = 256, 512, 256, 512, 10
NCORES = 8
BL = B // NCORES  # 32 batch rows per core
QT = 16           # timesteps per phase-1 chunk (512 cols = 32 b * 16 t)
NQ = T // QT      # 32 chunks

LAST_EXEC_NS = None

_BUILD_LOCK = threading.Lock()
_CACHED = {}


def _build_bass():
    import concourse.bass as bass
    import concourse.tile as tile
    from concourse import mybir
    from contextlib import ExitStack

    BF = mybir.dt.bfloat16
    F32 = mybir.dt.float32
    AF = mybir.ActivationFunctionType

    nc = bass.Bass()

    # ---- I/O -----------------------------------------------------------
    xT = nc.declare_dram_parameter("xT", [2, NQ, 128, 512], BF, isOutput=False)
    w_rec = nc.declare_dram_parameter("w_rec", [3, 4, 4, 128, 128], BF, isOutput=False)
    w_in = nc.declare_dram_parameter("w_in", [3, 2, 4, 128, 128], BF, isOutput=False)
    bias_gm = nc.declare_dram_parameter("bias_gm", [128, 12], F32, isOutput=False)
    fc_wT = nc.declare_dram_parameter("fc_wT", [4, 128, 10], F32, isOutput=False)
    fc_b = nc.declare_dram_parameter("fc_b", [10, 1], F32, isOutput=False)
    ident = nc.declare_dram_parameter("ident", [128, 128], BF, isOutput=False)
    out = nc.declare_dram_parameter("out", [10, BL], F32, isOutput=True)

    with tile.TileContext(nc) as tc, ExitStack() as ctx:
        consts = ctx.enter_context(tc.tile_pool(name="consts", bufs=1))

        # Resident weights / biases.
        wrec_sb = consts.tile([128, 3, 4, 4, 128], BF)
        nc.sync.dma_start(out=wrec_sb, in_=w_rec[:].rearrange("g k m p f -> p g k m f"))
        win_sb = consts.tile([128, 3, 2, 4, 128], BF)
        nc.sync.dma_start(out=win_sb, in_=w_in[:].rearrange("g k m p f -> p g k m f"))
        bias_sb = consts.tile([128, 12], F32)
        nc.sync.dma_start(out=bias_sb, in_=bias_gm[:])
        fcw_sb = consts.tile([128, 4, 10], F32)
        nc.sync.dma_start(out=fcw_sb, in_=fc_wT[:].rearrange("k p c -> p k c"))
        fcb_sb = consts.tile([10, 1], F32)
        nc.sync.dma_start(out=fcb_sb, in_=fc_b[:])
        ident_sb = consts.tile([128, 128], BF)
        nc.sync.dma_start(out=ident_sb, in_=ident[:])

        # State + pools.
        hpool = ctx.enter_context(tc.tile_pool(name="hstate", bufs=1))
        h_sb = hpool.tile([128, 128], BF)
        nc.vector.memset(h_sb, 0.0)

        upool = ctx.enter_context(tc.tile_pool(name="uring", bufs=3))
        xpool = ctx.enter_context(tc.tile_pool(name="p1x", bufs=3))
        ps1 = ctx.enter_context(tc.tile_pool(name="p1ps", bufs=2, space="PSUM"))
        ps2 = ctx.enter_context(tc.tile_pool(name="recps", bufs=1, space="PSUM"))
        work = ctx.enter_context(tc.tile_pool(name="work", bufs=2))

        # ---- Phase 1 as a stream of work items -------------------------
        # chunk q covers timesteps [16q, 16q+16); items are (g, m) groups:
        # two matmuls (k-chunks of F) + one bias-add/cast into the u ring.
        chunk_u = {}
        chunk_x = {}

        def p1_fetch_x(q):
            x0 = xpool.tile([128, 512], BF, tag="x0", name=f"x0_{q}")
            x1 = xpool.tile([128, 512], BF, tag="x1", name=f"x1_{q}")
            nc.sync.dma_start(out=x0, in_=xT[0, q])
            nc.sync.dma_start(out=x1, in_=xT[1, q])
            chunk_x[q] = (x0, x1)

        def p1_items(q):
            x0, x1 = chunk_x.pop(q)
            uts = []
            for g in range(3):
                ut = upool.tile([128, 4, QT, BL], BF, tag=f"u{g}", name=f"u{g}_{q}")
                uts.append(ut)
            chunk_u[q] = uts

            def make_item(g, m):
                def emit():
                    pp = ps1.tile([128, 512], F32, tag="pp", name=f"pp{q}_{g}{m}")
                    nc.tensor.matmul(pp, lhsT=win_sb[:, g, 0, m, :], rhs=x0,
                                     start=True, stop=False)
                    nc.tensor.matmul(pp, lhsT=win_sb[:, g, 1, m, :], rhs=x1,
                                     start=False, stop=True)
                    dst = chunk_u[q][g][:, m, :, :]
                    bap = bias_sb[:, g * 4 + m : g * 4 + m + 1]
                    ppv = pp.rearrange("p (tt b) -> p tt b", b=BL)
                    # all posts on DVE: they fit the DVE idle window after
                    # the blend, while an ACT post would delay sigma(r01).
                    nc.vector.tensor_scalar_add(dst, ppv, bap)
                return emit

            return [make_item(g, m) for m in range(4) for g in range(3)]

        pending = []
        for q in (0, 1, 2):
            p1_fetch_x(q)
        for q in (0, 1):
            for it in p1_items(q):
                it()

        # ---- Recurrence ------------------------------------------------
        for t in range(T):
            q, tt = divmod(t, QT)
            if tt == 0 and q + 2 < NQ:
                pending.extend(p1_items(q + 2))
            if tt == 8 and q + 3 < NQ:
                p1_fetch_x(q + 3)

            uz, ur, uh = chunk_u[q]

            # PSUM banks for this step.
            R01 = ps2.tile([128, 64], F32, tag="R01")
            R23 = ps2.tile([128, 64], F32, tag="R23")
            Zb = ps2.tile([128, 128], F32, tag="Zb")
            Cb = ps2.tile([128, 64], F32, tag="Cb")
            Db = ps2.tile([128, 64], F32, tag="Db")

            r_sb = work.tile([128, 128], BF, tag="r")
            z_sb = work.tile([128, 128], BF, tag="z")
            ht_sb = work.tile([128, 128], BF, tag="ht")
            rh_sb = work.tile([128, 128], BF, tag="rh")
            w_sb = work.tile([128, 128], BF, tag="w")
            e_sb = work.tile([128, 128], BF, tag="e")

            # u injection: identity matmul clears each bank (start=True)
            # and deposits the precomputed input contribution.
            nc.tensor.matmul(R01, lhsT=ident_sb, rhs=ur[:, 0:2, tt, :],
                             start=True, stop=False, skip_group_check=True)
            nc.tensor.matmul(R23, lhsT=ident_sb, rhs=ur[:, 2:4, tt, :],
                             start=True, stop=False, skip_group_check=True)
            nc.tensor.matmul(Zb, lhsT=ident_sb, rhs=uz[:, :, tt, :],
                             start=True, stop=False, skip_group_check=True)
            nc.tensor.matmul(Cb, lhsT=ident_sb, rhs=uh[:, 0:2, tt, :],
                             start=True, stop=False, skip_group_check=True)
            nc.tensor.matmul(Db, lhsT=ident_sb, rhs=uh[:, 2:4, tt, :],
                             start=True, stop=False, skip_group_check=True)

            # r gate, chunks 0,1 -> sigmoid overlaps chunks 2,3 matmuls.
            for m in (0, 1):
                for k in range(4):
                    nc.tensor.matmul(
                        R01[:, 32 * m : 32 * m + 32],
                        lhsT=wrec_sb[:, 1, k, m, :],
                        rhs=h_sb[:, 32 * k : 32 * k + 32],
                        start=False, stop=(k == 3), skip_group_check=True)
            nc.scalar.activation(r_sb[:, 0:64], R01, AF.Sigmoid)
            for m in (2, 3):
                for k in range(4):
                    nc.tensor.matmul(
                        R23[:, 32 * (m - 2) : 32 * (m - 2) + 32],
                        lhsT=wrec_sb[:, 1, k, m, :],
                        rhs=h_sb[:, 32 * k : 32 * k + 32],
                        start=False, stop=(k == 3), skip_group_check=True)
            nc.scalar.activation(r_sb[:, 64:128], R23, AF.Sigmoid)
            nc.vector.tensor_mul(rh_sb[:, 0:64], r_sb[:, 0:64], h_sb[:, 0:64])

            # z gate (single bank, sigmoid off the critical path).
            for m in range(4):
                for k in range(4):
                    nc.tensor.matmul(
                        Zb[:, 32 * m : 32 * m + 32],
                        lhsT=wrec_sb[:, 0, k, m, :],
                        rhs=h_sb[:, 32 * k : 32 * k + 32],
                        start=False, stop=(k == 3), skip_group_check=True)
            nc.scalar.activation(z_sb, Zb, AF.Sigmoid)
            nc.vector.tensor_mul(rh_sb[:, 64:128], r_sb[:, 64:128],
                                 h_sb[:, 64:128])

            # h_tilde: C bank holds chunks 0,1 of the output, D holds 2,3.
            # k-chunks 0,1 can start as soon as rh01 exists; C finishes
            # before D so tanh(C) overlaps D's last matmuls.
            def htmm(bank, ms, k, stop):
                for m in ms:
                    nc.tensor.matmul(
                        bank[:, 32 * (m % 2) : 32 * (m % 2) + 32],
                        lhsT=wrec_sb[:, 2, k, m, :],
                        rhs=rh_sb[:, 32 * k : 32 * k + 32],
                        start=False, stop=stop, skip_group_check=True)

            for k in (0, 1):
                htmm(Cb, (0, 1), k, False)
                htmm(Db, (2, 3), k, False)
            # off-path: w = (z-1)*h while the k=2,3 matmuls run.
            nc.vector.scalar_tensor_tensor(
                w_sb, z_sb, 1.0, h_sb,
                op0=mybir.AluOpType.subtract, op1=mybir.AluOpType.mult)
            htmm(Cb, (0, 1), 2, False)
            htmm(Cb, (0, 1), 3, True)
            nc.scalar.activation(ht_sb[:, 0:64], Cb, AF.Tanh)
            htmm(Db, (2, 3), 2, False)
            htmm(Db, (2, 3), 3, True)
            nc.scalar.activation(ht_sb[:, 64:128], Db, AF.Tanh)

            # blend h = z*ht - (z-1)*h in halves so next step's matmuls
            # start as soon as the first half lands.
            nc.vector.tensor_mul(e_sb[:, 0:64], z_sb[:, 0:64], ht_sb[:, 0:64])
            nc.vector.tensor_sub(h_sb[:, 0:64], e_sb[:, 0:64], w_sb[:, 0:64])
            nc.vector.tensor_mul(e_sb[:, 64:128], z_sb[:, 64:128],
                                 ht_sb[:, 64:128])
            nc.vector.tensor_sub(h_sb[:, 64:128], e_sb[:, 64:128],
                                 w_sb[:, 64:128])

            # phase-1 fill work: 12 items paced over the first 12 steps of
            # each 16-step window, keeping the chunk transition clean.
            if pending and tt < 12:
                pending.pop(0)()

        while pending:
            pending.pop(0)()

        # ---- Epilogue: logits.T = fc_w.T @ h + fc_b --------------------
        h_f32 = work.tile([128, 128], F32, tag="hf32")
        nc.vector.tensor_copy(h_f32, h_sb)
        with tc.tile_pool(name="psfc", bufs=1, space="PSUM") as psfc_pool:
            ps_fc = psfc_pool.tile([10, BL], F32)
            for k in range(4):
                nc.tensor.matmul(
                    ps_fc, lhsT=fcw_sb[:, k, :],
                    rhs=h_f32[:, 32 * k : 32 * k + 32],
                    start=(k == 0), stop=(k == 3))
            out_sb = consts.tile([10, BL], F32)
            nc.scalar.activation(out_sb, ps_fc, AF.Identity, bias=fcb_sb)
            nc.sync.dma_start(out=out[:], in_=out_sb)

    return nc


def _split_multi_waits(nc):
    """Walrus in this container accepts at most ONE embedded sem wait (and
    update) per instruction; Tile emits several.  Split the extras onto
    single-wait NoOps inserted just before (waits) / after (updates) the
    offending instruction on the same engine."""
    from concourse import mybir

    n_split = 0
    for fn in nc.m.functions:
        for blk in fn.blocks:
            insts = blk.instructions
            i = 0
            while i < len(insts):
                ins = insts[i]
                si = ins.sync_info
                if si is None:
                    i += 1
                    continue
                waits = list(si.on_wait)
                updates = list(si.on_update)
                if len(waits) <= 1 and len(updates) <= 1:
                    i += 1
                    continue
                for j, w in enumerate(waits[:-1]):
                    nop = mybir.InstNoOp(
                        name=f"{ins.name}-sw{j}",
                        engine=ins.engine,
                        sync_info=mybir.SyncInfo(on_wait=[w], on_update=[]),
                    )
                    insts.insert(i, nop)
                    i += 1
                for j, u in enumerate(updates[1:]):
                    nop = mybir.InstNoOp(
                        name=f"{ins.name}-su{j}",
                        engine=ins.engine,
                        sync_info=mybir.SyncInfo(on_wait=[], on_update=[u]),
                    )
                    insts.insert(i + 1, nop)
                ins.sync_info = mybir.SyncInfo(
                    on_wait=waits[-1:], on_update=updates[:1])
                n_split += 1
                i += 1 + len(updates[1:])
    return n_split


def _get_nc():
    with _BUILD_LOCK:
        if "nc" not in _CACHED:
            nc = _build_bass()
            _split_multi_waits(nc)
            _CACHED["nc"] = nc
        return _CACHED["nc"]


def _pack_inputs(x, Wz_w, Wz_b, Wr_w, Wr_b, Wh_w, Wh_b, fc_w, fc_b):
    """Host-side layout prep. Returns (shared dict, per-core xT list)."""
    gates_w = [Wz_w, Wr_w, Wh_w]
    gates_b = [Wz_b, Wr_b, Wh_b]

    w_rec = np.stack([
        w[:H].reshape(4, 128, 4, 128).transpose(0, 2, 1, 3) for w in gates_w
    ]).astype(BF16)
    w_in = np.stack([
        w[H:].reshape(2, 128, 4, 128).transpose(0, 2, 1, 3) for w in gates_w
    ]).astype(BF16)
    # bias_gm[p, g*4+m] = b_g[128*m + p]
    bias_gm = np.ascontiguousarray(
        np.stack(gates_b).reshape(3, 4, 128).transpose(2, 0, 1).reshape(128, 12)
    ).astype(np.float32)
    fc_wT = np.ascontiguousarray(fc_w.reshape(4, 128, C)).astype(np.float32)
    fc_bT = np.ascontiguousarray(fc_b.reshape(C, 1)).astype(np.float32)

    shared = {
        "w_rec": w_rec, "w_in": w_in, "bias_gm": bias_gm,
        "fc_wT": fc_wT, "fc_b": fc_bT,
        "ident": np.eye(128, dtype=BF16),
    }

    xTs = []
    for c in range(NCORES):
        xc = x[c * BL : (c + 1) * BL]  # [32, 512, 256]
        # xT[k, q, p, 32*tt + b] = xc[b, 16*q + tt, 128*k + p]
        arr = xc.reshape(BL, NQ, QT, 2, 128).transpose(3, 1, 4, 2, 0)
        xTs.append(np.ascontiguousarray(arr.reshape(2, NQ, 128, 512)).astype(BF16))
    return shared, xTs


def kernel(x, Wz_w, Wz_b, Wr_w, Wr_b, Wh_w, Wh_b, fc_w, fc_b):
    global LAST_EXEC_NS
    from concourse.bass_utils import run_bass_kernel_spmd

    x = np.asarray(x, dtype=np.float32)
    shared, xTs = _pack_inputs(
        x, np.asarray(Wz_w), np.asarray(Wz_b), np.asarray(Wr_w),
        np.asarray(Wr_b), np.asarray(Wh_w), np.asarray(Wh_b),
        np.asarray(fc_w), np.asarray(fc_b))

    nc = _get_nc()
    in_maps = [dict(shared, xT=xTs[c]) for c in range(NCORES)]
    trace = bool(int(os.environ.get("GRU_TRACE", "0")))
    res = run_bass_kernel_spmd(nc, in_maps, list(range(NCORES)), trace=trace)
    LAST_EXEC_NS = res.exec_time_ns

    outs = [res.results[c]["out"] for c in range(NCORES)]  # [10, 32] each
    logits = np.concatenate([o.T for o in outs], axis=0).astype(np.float32)
    return logits


# revision 10
# speedup vs baseline: 1.0497x; 1.0497x over previous
"""Trainium2 Bass kernel for the MinGRU (full-GRU) problem.

Shapes (hardcoded): x [256, 512, 256], W*_w [768, 512], W*_b [512],
fc_w [512, 10], fc_b [10, 1].  Output [256, 10] fp32.

Strategy: data-parallel over batch across 8 cores (B_local = 32).
Per core, everything lives in a transposed tile layout
(partition = feature-within-128-chunk, column = 32*chunk + batch):

  Phase 1 (interleaved): U_g = x @ W_g[H:] + b_g for all t, computed
      chunk-by-chunk (16 timesteps each) directly into an SBUF ring —
      no DRAM roundtrip.  Its matmuls/activations are emitted into the
      tail of each recurrence step so they fill engine idle slots.

  Recurrence (per step), scheduled to minimize the serial chain
      sigma(r) -> r*h -> h_tilde matmul -> tanh -> blend:
      - r-gate matmuls first, split across two PSUM banks so the first
        sigmoid (chunks 0,1) overlaps the second half's matmuls;
      - r*h computed per half, h_tilde matmuls start as soon as the
        k=0,1 chunks of r*h exist;
      - z-gate matmuls (single bank, single sigmoid) run in the shadow
        of the sigma(r)/r*h chain; (1-z)*h is precomputed off-path;
      - single tanh over the whole h_tilde bank; the blend
        h = (h - z*h) + z*h_tilde is split in column halves so the next
        step's k=0,1 matmuls can start before the second half lands.
"""

import os
import sys
import threading

import numpy as np

sys.path.insert(0, "/opt/trn_rl_repo")

import ml_dtypes

BF16 = ml_dtypes.bfloat16

B, T, F, H, C = 256, 512, 256, 512, 10
NCORES = 8
BL = B // NCORES  # 32 batch rows per core
QT = 16           # timesteps per phase-1 chunk (512 cols = 32 b * 16 t)
NQ = T // QT      # 32 chunks

LAST_EXEC_NS = None

_BUILD_LOCK = threading.Lock()
_CACHED = {}


def _build_bass():
    import concourse.bass as bass
    import concourse.tile as tile
    from concourse import mybir
    from contextlib import ExitStack

    BF = mybir.dt.bfloat16
    F32 = mybir.dt.float32
    AF = mybir.ActivationFunctionType

    nc = bass.Bass()

    # ---- I/O -----------------------------------------------------------
    xT = nc.declare_dram_parameter("xT", [2, NQ, 128, 512], BF, isOutput=False)
    w_rec = nc.declare_dram_parameter("w_rec", [3, 4, 4, 128, 128], BF, isOutput=False)
    w_in = nc.declare_dram_parameter("w_in", [3, 2, 4, 128, 128], BF, isOutput=False)
    bias_gm = nc.declare_dram_parameter("bias_gm", [128, 12], F32, isOutput=False)
    fc_wT = nc.declare_dram_parameter("fc_wT", [4, 128, 10], F32, isOutput=False)
    fc_b = nc.declare_dram_parameter("fc_b", [10, 1], F32, isOutput=False)
    ident = nc.declare_dram_parameter("ident", [128, 128], BF, isOutput=False)
    out = nc.declare_dram_parameter("out", [10, BL], F32, isOutput=True)

    with tile.TileContext(nc) as tc, ExitStack() as ctx:
        consts = ctx.enter_context(tc.tile_pool(name="consts", bufs=1))

        # Resident weights / biases.
        wrec_sb = consts.tile([128, 3, 4, 4, 128], BF)
        nc.sync.dma_start(out=wrec_sb, in_=w_rec[:].rearrange("g k m p f -> p g k m f"))
        win_sb = consts.tile([128, 3, 2, 4, 128], BF)
        nc.sync.dma_start(out=win_sb, in_=w_in[:].rearrange("g k m p f -> p g k m f"))
        bias_sb = consts.tile([128, 12], F32)
        nc.sync.dma_start(out=bias_sb, in_=bias_gm[:])
        fcw_sb = consts.tile([128, 4, 10], F32)
        nc.sync.dma_start(out=fcw_sb, in_=fc_wT[:].rearrange("k p c -> p k c"))
        fcb_sb = consts.tile([10, 1], F32)
        nc.sync.dma_start(out=fcb_sb, in_=fc_b[:])
        ident_sb = consts.tile([128, 128], BF)
        nc.sync.dma_start(out=ident_sb, in_=ident[:])

        # State + pools.
        hpool = ctx.enter_context(tc.tile_pool(name="hstate", bufs=1))
        h_sb = hpool.tile([128, 128], BF)
        nc.vector.memset(h_sb, 0.0)

        upool = ctx.enter_context(tc.tile_pool(name="uring", bufs=3))
        xpool = ctx.enter_context(tc.tile_pool(name="p1x", bufs=3))
        ps1 = ctx.enter_context(tc.tile_pool(name="p1ps", bufs=2, space="PSUM"))
        ps2 = ctx.enter_context(tc.tile_pool(name="recps", bufs=1, space="PSUM"))
        work = ctx.enter_context(tc.tile_pool(name="work", bufs=2))

        # ---- Phase 1 as a stream of work items -------------------------
        # chunk q covers timesteps [16q, 16q+16); items are (g, m) groups:
        # two matmuls (k-chunks of F) + one bias-add/cast into the u ring.
        chunk_u = {}
        chunk_x = {}

        def p1_fetch_x(q):
            x0 = xpool.tile([128, 512], BF, tag="x0", name=f"x0_{q}")
            x1 = xpool.tile([128, 512], BF, tag="x1", name=f"x1_{q}")
            nc.sync.dma_start(out=x0, in_=xT[0, q])
            nc.sync.dma_start(out=x1, in_=xT[1, q])
            chunk_x[q] = (x0, x1)

        def p1_items(q):
            x0, x1 = chunk_x.pop(q)
            uts = []
            for g in range(3):
                ut = upool.tile([128, 4, QT, BL], BF, tag=f"u{g}", name=f"u{g}_{q}")
                uts.append(ut)
            chunk_u[q] = uts

            def make_item(g, m):
                def emit():
                    pp = ps1.tile([128, 512], F32, tag="pp", name=f"pp{q}_{g}{m}")
                    nc.tensor.matmul(pp, lhsT=win_sb[:, g, 0, m, :], rhs=x0,
                                     start=True, stop=False)
                    nc.tensor.matmul(pp, lhsT=win_sb[:, g, 1, m, :], rhs=x1,
                                     start=False, stop=True)
                    dst = chunk_u[q][g][:, m, :, :]
                    bap = bias_sb[:, g * 4 + m : g * 4 + m + 1]
                    ppv = pp.rearrange("p (tt b) -> p tt b", b=BL)
                    # split bias+cast across ACT and DVE so neither engine
                    # becomes the bottleneck; m-major item order spreads the
                    # ACT posts one per three steps.
                    if g == 2:
                        nc.scalar.activation(dst, ppv, AF.Identity, bias=bap)
                    else:
                        nc.vector.tensor_scalar_add(dst, ppv, bap)
                return emit

            return [make_item(g, m) for m in range(4) for g in range(3)]

        pending = []
        for q in (0, 1, 2):
            p1_fetch_x(q)
        for q in (0, 1):
            for it in p1_items(q):
                it()

        # ---- Recurrence ------------------------------------------------
        for t in range(T):
            q, tt = divmod(t, QT)
            if tt == 0 and q + 2 < NQ:
                pending.extend(p1_items(q + 2))
            if tt == 8 and q + 3 < NQ:
                p1_fetch_x(q + 3)

            uz, ur, uh = chunk_u[q]

            # PSUM banks for this step.
            R01 = ps2.tile([128, 64], F32, tag="R01")
            R23 = ps2.tile([128, 64], F32, tag="R23")
            Zb = ps2.tile([128, 128], F32, tag="Zb")
            Cb = ps2.tile([128, 64], F32, tag="Cb")
            Db = ps2.tile([128, 64], F32, tag="Db")

            r_sb = work.tile([128, 128], BF, tag="r")
            z_sb = work.tile([128, 128], BF, tag="z")
            ht_sb = work.tile([128, 128], BF, tag="ht")
            rh_sb = work.tile([128, 128], BF, tag="rh")
            w_sb = work.tile([128, 128], BF, tag="w")
            e_sb = work.tile([128, 128], BF, tag="e")

            # u injection: identity matmul clears each bank (start=True)
            # and deposits the precomputed input contribution.
            nc.tensor.matmul(R01, lhsT=ident_sb, rhs=ur[:, 0:2, tt, :],
                             start=True, stop=False, skip_group_check=True)
            nc.tensor.matmul(R23, lhsT=ident_sb, rhs=ur[:, 2:4, tt, :],
                             start=True, stop=False, skip_group_check=True)
            nc.tensor.matmul(Zb, lhsT=ident_sb, rhs=uz[:, :, tt, :],
                             start=True, stop=False, skip_group_check=True)
            nc.tensor.matmul(Cb, lhsT=ident_sb, rhs=uh[:, 0:2, tt, :],
                             start=True, stop=False, skip_group_check=True)
            nc.tensor.matmul(Db, lhsT=ident_sb, rhs=uh[:, 2:4, tt, :],
                             start=True, stop=False, skip_group_check=True)

            # r gate, chunks 0,1 -> sigmoid overlaps chunks 2,3 matmuls.
            for m in (0, 1):
                for k in range(4):
                    nc.tensor.matmul(
                        R01[:, 32 * m : 32 * m + 32],
                        lhsT=wrec_sb[:, 1, k, m, :],
                        rhs=h_sb[:, 32 * k : 32 * k + 32],
                        start=False, stop=(k == 3), skip_group_check=True)
            nc.scalar.activation(r_sb[:, 0:64], R01, AF.Sigmoid)
            for m in (2, 3):
                for k in range(4):
                    nc.tensor.matmul(
                        R23[:, 32 * (m - 2) : 32 * (m - 2) + 32],
                        lhsT=wrec_sb[:, 1, k, m, :],
                        rhs=h_sb[:, 32 * k : 32 * k + 32],
                        start=False, stop=(k == 3), skip_group_check=True)
            nc.scalar.activation(r_sb[:, 64:128], R23, AF.Sigmoid)
            nc.vector.tensor_mul(rh_sb[:, 0:64], r_sb[:, 0:64], h_sb[:, 0:64])

            # z gate (single bank, sigmoid off the critical path).
            for m in range(4):
                for k in range(4):
                    nc.tensor.matmul(
                        Zb[:, 32 * m : 32 * m + 32],
                        lhsT=wrec_sb[:, 0, k, m, :],
                        rhs=h_sb[:, 32 * k : 32 * k + 32],
                        start=False, stop=(k == 3), skip_group_check=True)
            nc.scalar.activation(z_sb, Zb, AF.Sigmoid)
            nc.vector.tensor_mul(rh_sb[:, 64:128], r_sb[:, 64:128],
                                 h_sb[:, 64:128])

            # h_tilde: C bank holds chunks 0,1 of the output, D holds 2,3.
            # k-chunks 0,1 can start as soon as rh01 exists; C finishes
            # before D so tanh(C) overlaps D's last matmuls.
            def htmm(bank, ms, k, stop):
                for m in ms:
                    nc.tensor.matmul(
                        bank[:, 32 * (m % 2) : 32 * (m % 2) + 32],
                        lhsT=wrec_sb[:, 2, k, m, :],
                        rhs=rh_sb[:, 32 * k : 32 * k + 32],
                        start=False, stop=stop, skip_group_check=True)

            for k in (0, 1):
                htmm(Cb, (0, 1), k, False)
                htmm(Db, (2, 3), k, False)
            # off-path: w = (z-1)*h while the k=2,3 matmuls run.
            nc.vector.scalar_tensor_tensor(
                w_sb, z_sb, 1.0, h_sb,
                op0=mybir.AluOpType.subtract, op1=mybir.AluOpType.mult)
            htmm(Cb, (0, 1), 2, False)
            htmm(Cb, (0, 1), 3, True)
            nc.scalar.activation(ht_sb[:, 0:64], Cb, AF.Tanh)
            htmm(Db, (2, 3), 2, False)
            htmm(Db, (2, 3), 3, True)
            nc.scalar.activation(ht_sb[:, 64:128], Db, AF.Tanh)

            # blend h = z*ht - (z-1)*h in halves so next step's matmuls
            # start as soon as the first half lands.
            nc.vector.tensor_mul(e_sb[:, 0:64], z_sb[:, 0:64], ht_sb[:, 0:64])
            nc.vector.tensor_sub(h_sb[:, 0:64], e_sb[:, 0:64], w_sb[:, 0:64])
            nc.vector.tensor_mul(e_sb[:, 64:128], z_sb[:, 64:128],
                                 ht_sb[:, 64:128])
            nc.vector.tensor_sub(h_sb[:, 64:128], e_sb[:, 64:128],
                                 w_sb[:, 64:128])

            # phase-1 fill work: 12 items paced over the first 12 steps of
            # each 16-step window, keeping the chunk transition clean.
            if pending and tt < 12:
                pending.pop(0)()

        while pending:
            pending.pop(0)()

        # ---- Epilogue: logits.T = fc_w.T @ h + fc_b --------------------
        h_f32 = work.tile([128, 128], F32, tag="hf32")
        nc.vector.tensor_copy(h_f32, h_sb)
        with tc.tile_pool(name="psfc", bufs=1, space="PSUM") as psfc_pool:
            ps_fc = psfc_pool.tile([10, BL], F32)
            for k in range(4):
                nc.tensor.matmul(
                    ps_fc, lhsT=fcw_sb[:, k, :],
                    rhs=h_f32[:, 32 * k : 32 * k + 32],
                    start=(k == 0), stop=(k == 3))
            out_sb = consts.tile([10, BL], F32)
            nc.scalar.activation(out_sb, ps_fc, AF.Identity, bias=fcb_sb)
            nc.sync.dma_start(out=out[:], in_=out_sb)

    return nc


def _split_multi_waits(nc):
    """Walrus in this container accepts at most ONE embedded sem wait (and
    update) per instruction; Tile emits several.  Split the extras onto
    single-wait NoOps inserted just before (waits) / after (updates) the
    offending instruction on the same engine."""
    from concourse import mybir

    n_split = 0
    for fn in nc.m.functions:
        for blk in fn.blocks:
            insts = blk.instructions
            i = 0
            while i < len(insts):
                ins = insts[i]
                si = ins.sync_info
                if si is None:
                    i += 1
                    continue
                waits = list(si.on_wait)
                updates = list(si.on_update)
                if len(waits) <= 1 and len(updates) <= 1:
                    i += 1
                    continue
                for j, w in enumerate(waits[:-1]):
                    nop = mybir.InstNoOp(
                        name=f"{ins.name}-sw{j}",
                        engine=ins.engine,
                        sync_info=mybir.SyncInfo(on_wait=[w], on_update=[]),
                    )
                    insts.insert(i, nop)
                    i += 1
                for j, u in enumerate(updates[1:]):
                    nop = mybir.InstNoOp(
                        name=f"{ins.name}-su{j}",
                        engine=ins.engine,
                        sync_info=mybir.SyncInfo(on_wait=[], on_update=[u]),
                    )
                    insts.insert(i + 1, nop)
                ins.sync_info = mybir.SyncInfo(
                    on_wait=waits[-1:], on_update=updates[:1])
                n_split += 1
                i += 1 + len(updates[1:])
    return n_split


def _get_nc():
    with _BUILD_LOCK:
        if "nc" not in _CACHED:
            nc = _build_bass()
            _split_multi_waits(nc)
            _CACHED["nc"] = nc
        return _CACHED["nc"]


def _pack_inputs(x, Wz_w, Wz_b, Wr_w, Wr_b, Wh_w, Wh_b, fc_w, fc_b):
    """Host-side layout prep. Returns (shared dict, per-core xT list)."""
    gates_w = [Wz_w, Wr_w, Wh_w]
    gates_b = [Wz_b, Wr_b, Wh_b]

    w_rec = np.stack([
        w[:H].reshape(4, 128, 4, 128).transpose(0, 2, 1, 3) for w in gates_w
    ]).astype(BF16)
    w_in = np.stack([
        w[H:].reshape(2, 128, 4, 128).transpose(0, 2, 1, 3) for w in gates_w
    ]).astype(BF16)
    # bias_gm[p, g*4+m] = b_g[128*m + p]
    bias_gm = np.ascontiguousarray(
        np.stack(gates_b).reshape(3, 4, 128).transpose(2, 0, 1).reshape(128, 12)
    ).astype(np.float32)
    fc_wT = np.ascontiguousarray(fc_w.reshape(4, 128, C)).astype(np.float32)
    fc_bT = np.ascontiguousarray(fc_b.reshape(C, 1)).astype(np.float32)

    shared = {
        "w_rec": w_rec, "w_in": w_in, "bias_gm": bias_gm,
        "fc_wT": fc_wT, "fc_b": fc_bT,
        "ident": np.eye(128, dtype=BF16),
    }

    xTs = []
    for c in range(NCORES):
        xc = x[c * BL : (c + 1) * BL]  # [32, 512, 256]
        # xT[k, q, p, 32*tt + b] = xc[b, 16*q + tt, 128*k + p]
        arr = xc.reshape(BL, NQ, QT, 2, 128).transpose(3, 1, 4, 2, 0)
        xTs.append(np.ascontiguousarray(arr.reshape(2, NQ, 128, 512)).astype(BF16))
    return shared, xTs


def kernel(x, Wz_w, Wz_b, Wr_w, Wr_b, Wh_w, Wh_b, fc_w, fc_b):
    global LAST_EXEC_NS
    from concourse.bass_utils import run_bass_kernel_spmd

    x = np.asarray(x, dtype=np.float32)
    shared, xTs = _pack_inputs(
        x, np.asarray(Wz_w), np.asarray(Wz_b), np.asarray(Wr_w),
        np.asarray(Wr_b), np.asarray(Wh_w), np.asarray(Wh_b),
        np.asarray(fc_w), np.asarray(fc_b))

    nc = _get_nc()
    in_maps = [dict(shared, xT=xTs[c]) for c in range(NCORES)]
    trace = bool(int(os.environ.get("GRU_TRACE", "0")))
    res = run_bass_kernel_spmd(nc, in_maps, list(range(NCORES)), trace=trace)
    LAST_EXEC_NS = res.exec_time_ns

    outs = [res.results[c]["out"] for c in range(NCORES)]  # [10, 32] each
    logits = np.concatenate([o.T for o in outs], axis=0).astype(np.float32)
    return logits
